# revision 24
# baseline (speedup 1.0000x reference)
"""Trainium2 Bass kernel for nn_AttLayer (4-head attention, softmax over queries).

Sharding: data-parallel over batch. 8 batch elements -> 8 NeuronCores, zero
collectives.

Key algebraic restructuring: with C=64 channels the attention is rank-65.
Folding the projections through the score/value contractions (bias rows
appended via the augmented-ones trick):

  R_h       = G_h^T-contracted input               G_h = Wk_aug_h @ Wq_aug_h^T
  scoresT_h = Xa^T R_h                             (= Xa^T G_h Xa, 65 x 65 G)
  es        = exp(SCALE * scoresT)                 row-sum den fused into the
                                                   exp activation (accum_out)
  xtr[j,c]  = XaT[j,c] / den[j]                    reciprocal folded into the
                                                   65-wide transposed input
  M2_h[c,i] = sum_j xtr[j,c] * es[j,i]             (65 x 1024)
  out2     += F_h^T @ M2_h                         F_h = Wv_aug_h @ Wout_h
  out       = out2 + b_out + x

G_h and F_h are computed on the host in f32 (exact). Everything on-chip is
bf16 matmuls with f32 PSUM accumulation; the exp/normalize core is the
critical path (ScalarEngine), so all other work is software-pipelined into
the per-j-tile chain steps of neighboring heads.
"""

import numpy as np
import ml_dtypes

import concourse.tile as tile
from concourse import bacc, mybir
from concourse.bass_utils import run_bass_kernel_spmd

NH = 4          # heads
D = 640         # per-head dim
C = 64          # channels
CA = C + 1      # augmented (ones row)
SEQ = 1024      # 32*32
SCALE = float(D) ** -0.5
N_CORES = 8
FP = mybir.dt.float32
BF = mybir.dt.bfloat16

JT = SEQ // 128     # 8 j-tiles (128 keys each)
IC = SEQ // 512     # 2 i-chunks (512 queries each)

AF = mybir.ActivationFunctionType
ALU = mybir.AluOpType


def _build():
    nc = bacc.Bacc(None, target_bir_lowering=False)
    xa = nc.declare_dram_parameter("xa", [CA, SEQ], BF, isOutput=False)
    xt = nc.declare_dram_parameter("xt", [128, JT, CA], BF, isOutput=False)
    xf = nc.declare_dram_parameter("xf", [C, SEQ], FP, isOutput=False)
    gt = nc.declare_dram_parameter("gt", [CA, NH, CA], BF, isOutput=False)
    ff = nc.declare_dram_parameter("ff", [CA, NH, C], BF, isOutput=False)
    bo = nc.declare_dram_parameter("bo", [C, 1], FP, isOutput=False)
    out = nc.declare_dram_parameter("out", [C, SEQ], FP, isOutput=True)

    with tile.TileContext(nc) as tc:
        with (
            tc.tile_pool(name="consts", bufs=1) as consts,
            tc.tile_pool(name="hpool", bufs=3) as hpool,
            tc.tile_pool(name="sc", bufs=2, space="PSUM") as sc_psum,
            tc.tile_pool(name="pm", bufs=2, space="PSUM") as pm_psum,
        ):
            xa_sb = consts.tile([CA, SEQ], BF)
            for ic in range(IC):
                nc.sync.dma_start(
                    out=xa_sb[:, ic * 512:(ic + 1) * 512],
                    in_=xa[:, ic * 512:(ic + 1) * 512],
                )
            gt_sb = consts.tile([CA, NH, CA], BF)
            nc.sync.dma_start(out=gt_sb[:], in_=gt[:, :, :])
            xt_sb = consts.tile([128, JT, CA], BF)
            nc.sync.dma_start(out=xt_sb[:], in_=xt[:, :, :])
            ff_sb = consts.tile([CA, NH, C], BF)
            nc.sync.dma_start(out=ff_sb[:], in_=ff[:, :, :])
            xf_sb = consts.tile([C, SEQ], FP)
            nc.sync.dma_start(out=xf_sb[:], in_=xf[:, :])
            bo_sb = consts.tile([C, 1], FP)
            nc.sync.dma_start(out=bo_sb[:], in_=bo[:, :])
            out_sb = consts.tile([C, SEQ], FP)
            o2acc = consts.tile([C, SEQ], FP)

            def emit_R_ic(h, ic, state):
                if ic == 0:
                    state = (
                        hpool.tile([CA, SEQ], BF, tag="R", name=f"R_{h}"),
                        pm_psum.tile([CA, SEQ], FP, tag="pm", name=f"rp_{h}"),
                    )
                R_sb, rps = state
                nc.tensor.matmul(
                    rps[:, ic * 512:(ic + 1) * 512],
                    lhsT=gt_sb[:, h, :],
                    rhs=xa_sb[:, ic * 512:(ic + 1) * 512],
                    start=True, stop=True,
                )
                nc.vector.tensor_copy(
                    out=R_sb[:, ic * 512:(ic + 1) * 512],
                    in_=rps[:, ic * 512:(ic + 1) * 512],
                )
                return state

            def emit_R(h):
                state = emit_R_ic(h, 0, None)
                state = emit_R_ic(h, 1, state)
                return state[0]

            def emit_M2_mms(mps, xtr, es, jt):
                for ic in range(IC):
                    nc.tensor.matmul(
                        mps[:, ic * 512:(ic + 1) * 512],
                        lhsT=xtr[:, jt, :],
                        rhs=es[:, jt, ic * 512:(ic + 1) * 512],
                        start=(jt == 0), stop=(jt == JT - 1),
                    )

            def emit_m2_conv(ph, pmps):
                pm2 = hpool.tile([CA, SEQ], BF, tag="m2", name=f"m2_{ph}")
                for ic in range(IC):
                    nc.vector.tensor_copy(
                        out=pm2[:, ic * 512:(ic + 1) * 512],
                        in_=pmps[:, ic * 512:(ic + 1) * 512],
                    )
                return pm2

            def emit_out2(h, m2):
                o2p = pm_psum.tile([CA, SEQ], FP, tag="pm", name=f"o2_{h}")
                for ic in range(IC):
                    nc.tensor.matmul(
                        o2p[:C, ic * 512:(ic + 1) * 512],
                        lhsT=ff_sb[:, h, :],
                        rhs=m2[:, ic * 512:(ic + 1) * 512],
                        start=True, stop=True,
                    )
                if h == 0:
                    nc.vector.tensor_copy(out=o2acc[:], in_=o2p[:C, :])
                elif h < NH - 1:
                    nc.vector.tensor_add(out=o2acc[:], in0=o2acc[:], in1=o2p[:C, :])
                else:
                    # final head: o2acc already holds heads 0-2 plus residual
                    for ic in range(IC):
                        sl = slice(ic * 512, (ic + 1) * 512)
                        nc.scalar.activation(
                            out=out_sb[:, sl],
                            in_=o2p[:C, sl],
                            func=AF.Identity,
                            bias=bo_sb[:, 0:1],
                            scale=1.0,
                        )
                        nc.vector.tensor_add(
                            out=out_sb[:, sl], in0=out_sb[:, sl], in1=o2acc[:, sl],
                        )
                        for q in range(2):
                            qsl = slice(ic * 512 + q * 256, ic * 512 + (q + 1) * 256)
                            nc.sync.dma_start(out=out[:, qsl], in_=out_sb[:, qsl])

            R_cur = emit_R(0)
            R_nxt = None
            prev = None   # (h, es, xtr, mps) of the previous head
            for h in range(NH):
                R_sb = R_cur
                last = h == NH - 1
                es = hpool.tile([128, JT, SEQ], BF, tag="es", name=f"es_{h}")
                xtr = hpool.tile([128, JT, CA], BF, tag="xtr", name=f"xtr_{h}")
                den = hpool.tile([128, JT], FP, tag="den", name=f"den_{h}")
                rec = hpool.tile([128, JT], FP, tag="rec", name=f"rec_{h}")
                own_mps = (
                    pm_psum.tile([CA, SEQ], FP, tag="pm", name="mp_last")
                    if last else None
                )

                for jt in range(JT):
                    pst = sc_psum.tile([128, SEQ], FP, tag="sc", name=f"sc_{h}_{jt}")
                    for ic in range(IC):
                        nc.tensor.matmul(
                            pst[:, ic * 512:(ic + 1) * 512],
                            lhsT=xa_sb[:, jt * 128:(jt + 1) * 128],
                            rhs=R_sb[:, ic * 512:(ic + 1) * 512],
                            start=True, stop=True,
                        )
                    nc.scalar.activation(
                        out=es[:, jt, :],
                        in_=pst[:],
                        func=AF.Exp,
                        scale=SCALE,
                        accum_out=den[:, jt:jt + 1],
                    )
                    nc.vector.reciprocal(out=rec[:, jt:jt + 1], in_=den[:, jt:jt + 1])
                    nc.vector.tensor_scalar_mul(
                        xtr[:, jt, :], xt_sb[:, jt, :], rec[:, jt:jt + 1],
                    )

                    # ---- pipelined injections (<=2 matmuls per chain step)
                    if prev is not None:
                        ph, pes, pxtr, pmps = prev
                        emit_M2_mms(pmps, pxtr, pes, jt)
                        if jt == JT - 1:
                            pm2 = emit_m2_conv(ph, pmps)
                            emit_out2(ph, pm2)
                            prev = None
                    if jt == 2 and h + 1 < NH:
                        R_state = emit_R_ic(h + 1, 0, None)
                    if jt == 3 and h + 1 < NH:
                        R_nxt = emit_R_ic(h + 1, 1, R_state)[0]
                    if last and jt >= 1:
                        emit_M2_mms(own_mps, xtr, es, jt - 1)
                    if last and jt == 7:
                        nc.vector.tensor_add(
                            out=o2acc[:], in0=o2acc[:], in1=xf_sb[:],
                        )

                if not last:
                    mps = pm_psum.tile([CA, SEQ], FP, tag="pm", name=f"mp_{h}")
                    prev = (h, es, xtr, mps)
                R_cur = R_nxt

            # drain the last head's M2 tail (j-tile 7) and final output
            emit_M2_mms(own_mps, xtr, es, 7)
            pm2 = emit_m2_conv(NH - 1, own_mps)
            emit_out2(NH - 1, pm2)

    nc.compile()
    return nc


_CACHE: dict = {}


def _get_nc():
    if "nc" not in _CACHE:
        _CACHE["nc"] = _build()
    return _CACHE["nc"]


def _prep_in_maps(x, W_proj, b_proj, W_out, b_out):
    bf = ml_dtypes.bfloat16
    x = np.ascontiguousarray(np.asarray(x, dtype=np.float32))
    W_proj = np.asarray(W_proj, dtype=np.float32)
    b_proj = np.asarray(b_proj, dtype=np.float32)
    W_out = np.asarray(W_out, dtype=np.float32)
    b_out = np.asarray(b_out, dtype=np.float32)

    x2 = x.reshape(N_CORES, C, SEQ)
    xa_all = np.empty((N_CORES, CA, SEQ), dtype=bf)
    xa_all[:, :C, :] = x2.astype(bf)
    xa_all[:, C, :] = np.float32(1.0)

    # XaT: [core][p, jt, c'] = x[c', jt*128+p], ones at c'=64
    xt_all = np.empty((N_CORES, 128, JT, CA), dtype=bf)
    xtt = x2.transpose(0, 2, 1).reshape(N_CORES, JT, 128, C)  # [b, jt, p, c]
    xt_all[:, :, :, :C] = xtt.transpose(0, 2, 1, 3).astype(bf)
    xt_all[:, :, :, C] = np.float32(1.0)

    # augmented per-head projection blocks [65, 640]
    Wa = np.concatenate([W_proj, b_proj[None, :]], axis=0)  # [65, 7680]
    gt = np.empty((CA, NH, CA), dtype=bf)
    ffm = np.empty((CA, NH, C), dtype=bf)
    for h in range(NH):
        q0 = h * 3 * D
        Wq = Wa[:, q0:q0 + D]            # [65, 640]
        Wk = Wa[:, q0 + D:q0 + 2 * D]
        Wv = Wa[:, q0 + 2 * D:q0 + 3 * D]
        G = Wk @ Wq.T                    # [65, 65]; scoresT = Xa^T G Xa
        gt[:, h, :] = G.T.astype(bf)     # lhsT[c', c] = G[c, c']
        F = Wv @ W_out[h * D:(h + 1) * D, :]   # [65, 64]
        ffm[:, h, :] = F.astype(bf)

    bo = np.ascontiguousarray(b_out.reshape(C, 1))

    return [
        {
            "xa": np.ascontiguousarray(xa_all[i]),
            "xt": np.ascontiguousarray(xt_all[i]),
            "xf": np.ascontiguousarray(x2[i]),
            "gt": gt,
            "ff": ffm,
            "bo": bo,
        }
        for i in range(N_CORES)
    ]


def run(x, t, W_proj, b_proj, W_out, b_out, trace=False, **trace_kwargs):
    in_maps = _prep_in_maps(x, W_proj, b_proj, W_out, b_out)
    res = run_bass_kernel_spmd(
        _get_nc(), in_maps, core_ids=list(range(N_CORES)),
        trace=trace, **trace_kwargs,
    )
    out = np.stack([res.results[i]["out"] for i in range(N_CORES)])
    return out.reshape(N_CORES, C, 32, 32), res


def kernel(x, t=None, W_proj=None, b_proj=None, W_out=None, b_out=None):
    out, _ = run(x, t, W_proj, b_proj, W_out, b_out, trace=False)
    return out


# revision 25
# speedup vs baseline: 1.0076x; 1.0076x over previous
"""Trainium2 Bass kernel for nn_AttLayer (4-head attention, softmax over queries).

Sharding: data-parallel over batch. 8 batch elements -> 8 NeuronCores, zero
collectives.

Key algebraic restructuring: with C=64 channels the attention is rank-65.
Folding the projections through the score/value contractions (bias rows
appended via the augmented-ones trick):

  R_h       = G_h^T-contracted input               G_h = Wk_aug_h @ Wq_aug_h^T
  scoresT_h = Xa^T R_h                             (= Xa^T G_h Xa, 65 x 65 G)
  es        = exp(SCALE * scoresT)                 row-sum den fused into the
                                                   exp activation (accum_out)
  xtr[j,c]  = XaT[j,c] / den[j]                    reciprocal folded into the
                                                   65-wide transposed input
  M2_h[c,i] = sum_j xtr[j,c] * es[j,i]             (65 x 1024)
  out2     += F_h^T @ M2_h                         F_h = Wv_aug_h @ Wout_h
  out       = out2 + b_out + x

G_h and F_h are computed on the host in f32 (exact). Everything on-chip is
bf16 matmuls with f32 PSUM accumulation; the exp/normalize core is the
critical path (ScalarEngine), so all other work is software-pipelined into
the per-j-tile chain steps of neighboring heads.
"""

import numpy as np
import ml_dtypes

import concourse.tile as tile
from concourse import bacc, mybir
from concourse.bass_utils import run_bass_kernel_spmd

NH = 4          # heads
D = 640         # per-head dim
C = 64          # channels
CA = C + 1      # augmented (ones row)
SEQ = 1024      # 32*32
SCALE = float(D) ** -0.5
N_CORES = 8
FP = mybir.dt.float32
BF = mybir.dt.bfloat16

JT = SEQ // 128     # 8 j-tiles (128 keys each)
IC = SEQ // 512     # 2 i-chunks (512 queries each)

AF = mybir.ActivationFunctionType
ALU = mybir.AluOpType


def _build():
    nc = bacc.Bacc(None, target_bir_lowering=False)
    xa = nc.declare_dram_parameter("xa", [CA, SEQ], BF, isOutput=False)
    xt = nc.declare_dram_parameter("xt", [128, JT, CA], BF, isOutput=False)
    xf = nc.declare_dram_parameter("xf", [C, SEQ], FP, isOutput=False)
    gt = nc.declare_dram_parameter("gt", [CA, NH, CA], BF, isOutput=False)
    ff = nc.declare_dram_parameter("ff", [CA, NH, C], BF, isOutput=False)
    bo = nc.declare_dram_parameter("bo", [C, 1], FP, isOutput=False)
    out = nc.declare_dram_parameter("out", [C, SEQ], FP, isOutput=True)

    with tile.TileContext(nc) as tc:
        with (
            tc.tile_pool(name="consts", bufs=1) as consts,
            tc.tile_pool(name="hpool", bufs=3) as hpool,
            tc.tile_pool(name="sc", bufs=2, space="PSUM") as sc_psum,
            tc.tile_pool(name="pm", bufs=2, space="PSUM") as pm_psum,
        ):
            xa_sb = consts.tile([CA, SEQ], BF)
            for ic in range(IC):
                nc.sync.dma_start(
                    out=xa_sb[:, ic * 512:(ic + 1) * 512],
                    in_=xa[:, ic * 512:(ic + 1) * 512],
                )
            gt_sb = consts.tile([CA, NH, CA], BF)
            nc.sync.dma_start(out=gt_sb[:], in_=gt[:, :, :])
            xt_sb = consts.tile([128, JT, CA], BF)
            ff_sb = consts.tile([CA, NH, C], BF)
            xf_sb = consts.tile([C, SEQ], FP)
            bo_sb = consts.tile([C, 1], FP)
            out_sb = consts.tile([C, SEQ], FP)
            o2acc = consts.tile([C, SEQ], FP)

            def emit_late_dmas():
                nc.sync.dma_start(out=xt_sb[:], in_=xt[:, :, :])
                nc.sync.dma_start(out=ff_sb[:], in_=ff[:, :, :])
                nc.sync.dma_start(out=xf_sb[:], in_=xf[:, :])
                nc.sync.dma_start(out=bo_sb[:], in_=bo[:, :])

            def emit_R_ic(h, ic, state):
                if ic == 0:
                    state = (
                        hpool.tile([CA, SEQ], BF, tag="R", name=f"R_{h}"),
                        pm_psum.tile([CA, SEQ], FP, tag="pm", name=f"rp_{h}"),
                    )
                R_sb, rps = state
                nc.tensor.matmul(
                    rps[:, ic * 512:(ic + 1) * 512],
                    lhsT=gt_sb[:, h, :],
                    rhs=xa_sb[:, ic * 512:(ic + 1) * 512],
                    start=True, stop=True,
                )
                nc.vector.tensor_copy(
                    out=R_sb[:, ic * 512:(ic + 1) * 512],
                    in_=rps[:, ic * 512:(ic + 1) * 512],
                )
                return state

            def emit_R(h):
                state = emit_R_ic(h, 0, None)
                state = emit_R_ic(h, 1, state)
                return state[0]

            def emit_M2_mms(mps, xtr, es, jt):
                for ic in range(IC):
                    nc.tensor.matmul(
                        mps[:, ic * 512:(ic + 1) * 512],
                        lhsT=xtr[:, jt, :],
                        rhs=es[:, jt, ic * 512:(ic + 1) * 512],
                        start=(jt == 0), stop=(jt == JT - 1),
                    )

            def emit_m2_conv(ph, pmps):
                pm2 = hpool.tile([CA, SEQ], BF, tag="m2", name=f"m2_{ph}")
                for ic in range(IC):
                    nc.vector.tensor_copy(
                        out=pm2[:, ic * 512:(ic + 1) * 512],
                        in_=pmps[:, ic * 512:(ic + 1) * 512],
                    )
                return pm2

            def emit_out2(h, m2):
                o2p = pm_psum.tile([CA, SEQ], FP, tag="pm", name=f"o2_{h}")
                for ic in range(IC):
                    nc.tensor.matmul(
                        o2p[:C, ic * 512:(ic + 1) * 512],
                        lhsT=ff_sb[:, h, :],
                        rhs=m2[:, ic * 512:(ic + 1) * 512],
                        start=True, stop=True,
                    )
                if h == 0:
                    nc.vector.tensor_copy(out=o2acc[:], in_=o2p[:C, :])
                elif h < NH - 1:
                    nc.vector.tensor_add(out=o2acc[:], in0=o2acc[:], in1=o2p[:C, :])
                else:
                    # final head: o2acc already holds heads 0-2 plus residual
                    for ic in range(IC):
                        sl = slice(ic * 512, (ic + 1) * 512)
                        nc.scalar.activation(
                            out=out_sb[:, sl],
                            in_=o2p[:C, sl],
                            func=AF.Identity,
                            bias=bo_sb[:, 0:1],
                            scale=1.0,
                        )
                        nc.vector.tensor_add(
                            out=out_sb[:, sl], in0=out_sb[:, sl], in1=o2acc[:, sl],
                        )
                        for q in range(2):
                            qsl = slice(ic * 512 + q * 256, ic * 512 + (q + 1) * 256)
                            nc.sync.dma_start(out=out[:, qsl], in_=out_sb[:, qsl])

            R_cur = emit_R(0)
            emit_late_dmas()
            R_nxt = None
            prev = None   # (h, es, xtr, mps) of the previous head
            for h in range(NH):
                R_sb = R_cur
                last = h == NH - 1
                es = hpool.tile([128, JT, SEQ], BF, tag="es", name=f"es_{h}")
                xtr = hpool.tile([128, JT, CA], BF, tag="xtr", name=f"xtr_{h}")
                den = hpool.tile([128, JT], FP, tag="den", name=f"den_{h}")
                rec = hpool.tile([128, JT], FP, tag="rec", name=f"rec_{h}")
                own_mps = (
                    pm_psum.tile([CA, SEQ], FP, tag="pm", name="mp_last")
                    if last else None
                )

                for jt in range(JT):
                    pst = sc_psum.tile([128, SEQ], FP, tag="sc", name=f"sc_{h}_{jt}")
                    for ic in range(IC):
                        nc.tensor.matmul(
                            pst[:, ic * 512:(ic + 1) * 512],
                            lhsT=xa_sb[:, jt * 128:(jt + 1) * 128],
                            rhs=R_sb[:, ic * 512:(ic + 1) * 512],
                            start=True, stop=True,
                        )
                    nc.scalar.activation(
                        out=es[:, jt, :],
                        in_=pst[:],
                        func=AF.Exp,
                        scale=SCALE,
                        accum_out=den[:, jt:jt + 1],
                    )
                    nc.vector.reciprocal(out=rec[:, jt:jt + 1], in_=den[:, jt:jt + 1])
                    nc.vector.tensor_scalar_mul(
                        xtr[:, jt, :], xt_sb[:, jt, :], rec[:, jt:jt + 1],
                    )

                    # ---- pipelined injections (<=2 matmuls per chain step)
                    if prev is not None:
                        ph, pes, pxtr, pmps = prev
                        emit_M2_mms(pmps, pxtr, pes, jt)
                        if jt == JT - 1:
                            pm2 = emit_m2_conv(ph, pmps)
                            emit_out2(ph, pm2)
                            prev = None
                    if jt == 2 and h + 1 < NH:
                        R_state = emit_R_ic(h + 1, 0, None)
                    if jt == 3 and h + 1 < NH:
                        R_nxt = emit_R_ic(h + 1, 1, R_state)[0]
                    if last and jt >= 1:
                        emit_M2_mms(own_mps, xtr, es, jt - 1)
                    if last and jt == 7:
                        nc.vector.tensor_add(
                            out=o2acc[:], in0=o2acc[:], in1=xf_sb[:],
                        )

                if not last:
                    mps = pm_psum.tile([CA, SEQ], FP, tag="pm", name=f"mp_{h}")
                    prev = (h, es, xtr, mps)
                R_cur = R_nxt

            # drain the last head's M2 tail (j-tile 7) and final output
            emit_M2_mms(own_mps, xtr, es, 7)
            pm2 = emit_m2_conv(NH - 1, own_mps)
            emit_out2(NH - 1, pm2)

    nc.compile()
    return nc


_CACHE: dict = {}


def _get_nc():
    if "nc" not in _CACHE:
        _CACHE["nc"] = _build()
    return _CACHE["nc"]


def _prep_in_maps(x, W_proj, b_proj, W_out, b_out):
    bf = ml_dtypes.bfloat16
    x = np.ascontiguousarray(np.asarray(x, dtype=np.float32))
    W_proj = np.asarray(W_proj, dtype=np.float32)
    b_proj = np.asarray(b_proj, dtype=np.float32)
    W_out = np.asarray(W_out, dtype=np.float32)
    b_out = np.asarray(b_out, dtype=np.float32)

    x2 = x.reshape(N_CORES, C, SEQ)
    xa_all = np.empty((N_CORES, CA, SEQ), dtype=bf)
    xa_all[:, :C, :] = x2.astype(bf)
    xa_all[:, C, :] = np.float32(1.0)

    # XaT: [core][p, jt, c'] = x[c', jt*128+p], ones at c'=64
    xt_all = np.empty((N_CORES, 128, JT, CA), dtype=bf)
    xtt = x2.transpose(0, 2, 1).reshape(N_CORES, JT, 128, C)  # [b, jt, p, c]
    xt_all[:, :, :, :C] = xtt.transpose(0, 2, 1, 3).astype(bf)
    xt_all[:, :, :, C] = np.float32(1.0)

    # augmented per-head projection blocks [65, 640]
    Wa = np.concatenate([W_proj, b_proj[None, :]], axis=0)  # [65, 7680]
    gt = np.empty((CA, NH, CA), dtype=bf)
    ffm = np.empty((CA, NH, C), dtype=bf)
    for h in range(NH):
        q0 = h * 3 * D
        Wq = Wa[:, q0:q0 + D]            # [65, 640]
        Wk = Wa[:, q0 + D:q0 + 2 * D]
        Wv = Wa[:, q0 + 2 * D:q0 + 3 * D]
        G = Wk @ Wq.T                    # [65, 65]; scoresT = Xa^T G Xa
        gt[:, h, :] = G.T.astype(bf)     # lhsT[c', c] = G[c, c']
        F = Wv @ W_out[h * D:(h + 1) * D, :]   # [65, 64]
        ffm[:, h, :] = F.astype(bf)

    bo = np.ascontiguousarray(b_out.reshape(C, 1))

    return [
        {
            "xa": np.ascontiguousarray(xa_all[i]),
            "xt": np.ascontiguousarray(xt_all[i]),
            "xf": np.ascontiguousarray(x2[i]),
            "gt": gt,
            "ff": ffm,
            "bo": bo,
        }
        for i in range(N_CORES)
    ]


def run(x, t, W_proj, b_proj, W_out, b_out, trace=False, **trace_kwargs):
    in_maps = _prep_in_maps(x, W_proj, b_proj, W_out, b_out)
    res = run_bass_kernel_spmd(
        _get_nc(), in_maps, core_ids=list(range(N_CORES)),
        trace=trace, **trace_kwargs,
    )
    out = np.stack([res.results[i]["out"] for i in range(N_CORES)])
    return out.reshape(N_CORES, C, 32, 32), res


def kernel(x, t=None, W_proj=None, b_proj=None, W_out=None, b_out=None):
    out, _ = run(x, t, W_proj, b_proj, W_out, b_out, trace=False)
    return out


# revision 26
# speedup vs baseline: 1.0259x; 1.0182x over previous
"""Trainium2 Bass kernel for nn_AttLayer (4-head attention, softmax over queries).

Sharding: data-parallel over batch. 8 batch elements -> 8 NeuronCores, zero
collectives.

Key algebraic restructuring: with C=64 channels the attention is rank-65.
Folding the projections through the score/value contractions (bias rows
appended via the augmented-ones trick):

  R_h       = G_h^T-contracted input               G_h = Wk_aug_h @ Wq_aug_h^T
  scoresT_h = Xa^T R_h                             (= Xa^T G_h Xa, 65 x 65 G)
  es        = exp(SCALE * scoresT)                 row-sum den fused into the
                                                   exp activation (accum_out)
  xtr[j,c]  = XaT[j,c] / den[j]                    reciprocal folded into the
                                                   65-wide transposed input
  M2_h[c,i] = sum_j xtr[j,c] * es[j,i]             (65 x 1024)
  out2     += F_h^T @ M2_h                         F_h = Wv_aug_h @ Wout_h
  out       = out2 + b_out + x

G_h and F_h are computed on the host in f32 (exact). Everything on-chip is
bf16 matmuls with f32 PSUM accumulation; the exp/normalize core is the
critical path (ScalarEngine), so all other work is software-pipelined into
the per-j-tile chain steps of neighboring heads.
"""

import numpy as np
import ml_dtypes

import concourse.tile as tile
from concourse import bacc, mybir
from concourse.bass_utils import run_bass_kernel_spmd

NH = 4          # heads
D = 640         # per-head dim
C = 64          # channels
CA = C + 1      # augmented (ones row)
SEQ = 1024      # 32*32
SCALE = float(D) ** -0.5
N_CORES = 8
FP = mybir.dt.float32
BF = mybir.dt.bfloat16

JT = SEQ // 128     # 8 j-tiles (128 keys each)
IC = SEQ // 512     # 2 i-chunks (512 queries each)

AF = mybir.ActivationFunctionType
ALU = mybir.AluOpType


def _build():
    nc = bacc.Bacc(None, target_bir_lowering=False)
    # packed input blobs: one DMA each (descriptor setup dominates small DMAs)
    W_QA = SEQ + NH * CA + NH * C          # xa | gt | ff   on rows 0..64
    qa = nc.declare_dram_parameter("qa", [CA, W_QA], BF, isOutput=False)
    xt = nc.declare_dram_parameter("xt", [128, JT * CA], BF, isOutput=False)
    xr = nc.declare_dram_parameter("xr", [C, SEQ + 1], FP, isOutput=False)
    out = nc.declare_dram_parameter("out", [C, SEQ], FP, isOutput=True)

    with tile.TileContext(nc) as tc:
        with (
            tc.tile_pool(name="consts", bufs=1) as consts,
            tc.tile_pool(name="hpool", bufs=3) as hpool,
            tc.tile_pool(name="sc", bufs=2, space="PSUM") as sc_psum,
            tc.tile_pool(name="pm", bufs=2, space="PSUM") as pm_psum,
        ):
            qa_sb = consts.tile([CA, W_QA], BF)
            nc.sync.dma_start(out=qa_sb[:], in_=qa[:, :])
            xtb_sb = consts.tile([128, JT * CA], BF)
            xr_sb = consts.tile([C, SEQ + 1], FP)
            xa_sb = qa_sb[:, 0:SEQ]

            def gt_view(h):
                return qa_sb[:, SEQ + h * CA: SEQ + (h + 1) * CA]

            def ff_view(h):
                return qa_sb[:, SEQ + NH * CA + h * C: SEQ + NH * CA + (h + 1) * C]

            def xt_view(jt):
                return xtb_sb[:, jt * CA:(jt + 1) * CA]

            xf_sb = xr_sb[:, 0:SEQ]
            bo_sb = xr_sb[:, SEQ:SEQ + 1]
            out_sb = consts.tile([C, SEQ], FP)
            o2acc = consts.tile([C, SEQ], FP)

            def emit_late_dmas():
                nc.sync.dma_start(out=xtb_sb[:], in_=xt[:, :])
                nc.sync.dma_start(out=xr_sb[:], in_=xr[:, :])

            def emit_R_ic(h, ic, state):
                if ic == 0:
                    state = (
                        hpool.tile([CA, SEQ], BF, tag="R", name=f"R_{h}"),
                        pm_psum.tile([CA, SEQ], FP, tag="pm", name=f"rp_{h}"),
                    )
                R_sb, rps = state
                nc.tensor.matmul(
                    rps[:, ic * 512:(ic + 1) * 512],
                    lhsT=gt_view(h),
                    rhs=xa_sb[:, ic * 512:(ic + 1) * 512],
                    start=True, stop=True,
                )
                nc.vector.tensor_copy(
                    out=R_sb[:, ic * 512:(ic + 1) * 512],
                    in_=rps[:, ic * 512:(ic + 1) * 512],
                )
                return state

            def emit_R(h):
                state = emit_R_ic(h, 0, None)
                state = emit_R_ic(h, 1, state)
                return state[0]

            def emit_M2_mms(mps, xtr, es, jt):
                for ic in range(IC):
                    nc.tensor.matmul(
                        mps[:, ic * 512:(ic + 1) * 512],
                        lhsT=xtr[:, jt, :],
                        rhs=es[:, jt, ic * 512:(ic + 1) * 512],
                        start=(jt == 0), stop=(jt == JT - 1),
                    )

            def emit_m2_conv(ph, pmps):
                pm2 = hpool.tile([CA, SEQ], BF, tag="m2", name=f"m2_{ph}")
                for ic in range(IC):
                    nc.vector.tensor_copy(
                        out=pm2[:, ic * 512:(ic + 1) * 512],
                        in_=pmps[:, ic * 512:(ic + 1) * 512],
                    )
                return pm2

            def emit_out2(h, m2):
                o2p = pm_psum.tile([CA, SEQ], FP, tag="pm", name=f"o2_{h}")
                for ic in range(IC):
                    nc.tensor.matmul(
                        o2p[:C, ic * 512:(ic + 1) * 512],
                        lhsT=ff_view(h),
                        rhs=m2[:, ic * 512:(ic + 1) * 512],
                        start=True, stop=True,
                    )
                if h == 0:
                    nc.vector.tensor_copy(out=o2acc[:], in_=o2p[:C, :])
                elif h < NH - 1:
                    nc.vector.tensor_add(out=o2acc[:], in0=o2acc[:], in1=o2p[:C, :])
                else:
                    # final head: o2acc already holds heads 0-2 plus residual
                    for ic in range(IC):
                        sl = slice(ic * 512, (ic + 1) * 512)
                        nc.scalar.activation(
                            out=out_sb[:, sl],
                            in_=o2p[:C, sl],
                            func=AF.Identity,
                            bias=bo_sb[:],
                            scale=1.0,
                        )
                        nc.vector.tensor_add(
                            out=out_sb[:, sl], in0=out_sb[:, sl], in1=o2acc[:, sl],
                        )
                        for q in range(2):
                            qsl = slice(ic * 512 + q * 256, ic * 512 + (q + 1) * 256)
                            nc.sync.dma_start(out=out[:, qsl], in_=out_sb[:, qsl])

            R_cur = emit_R(0)
            emit_late_dmas()
            R_nxt = None
            prev = None   # (h, es, xtr, mps) of the previous head
            for h in range(NH):
                R_sb = R_cur
                last = h == NH - 1
                es = hpool.tile([128, JT, SEQ], BF, tag="es", name=f"es_{h}")
                xtr = hpool.tile([128, JT, CA], BF, tag="xtr", name=f"xtr_{h}")
                den = hpool.tile([128, JT], FP, tag="den", name=f"den_{h}")
                rec = hpool.tile([128, JT], FP, tag="rec", name=f"rec_{h}")
                own_mps = (
                    pm_psum.tile([CA, SEQ], FP, tag="pm", name="mp_last")
                    if last else None
                )

                for jt in range(JT):
                    pst = sc_psum.tile([128, SEQ], FP, tag="sc", name=f"sc_{h}_{jt}")
                    for ic in range(IC):
                        nc.tensor.matmul(
                            pst[:, ic * 512:(ic + 1) * 512],
                            lhsT=xa_sb[:, jt * 128:(jt + 1) * 128],
                            rhs=R_sb[:, ic * 512:(ic + 1) * 512],
                            start=True, stop=True,
                        )
                    nc.scalar.activation(
                        out=es[:, jt, :],
                        in_=pst[:],
                        func=AF.Exp,
                        scale=SCALE,
                        accum_out=den[:, jt:jt + 1],
                    )
                    nc.vector.reciprocal(out=rec[:, jt:jt + 1], in_=den[:, jt:jt + 1])
                    nc.vector.tensor_scalar_mul(
                        xtr[:, jt, :], xt_view(jt), rec[:, jt:jt + 1],
                    )

                    # ---- pipelined injections (<=2 matmuls per chain step)
                    if prev is not None:
                        ph, pes, pxtr, pmps = prev
                        emit_M2_mms(pmps, pxtr, pes, jt)
                        if jt == JT - 1:
                            pm2 = emit_m2_conv(ph, pmps)
                            emit_out2(ph, pm2)
                            prev = None
                    if jt == 2 and h + 1 < NH:
                        R_state = emit_R_ic(h + 1, 0, None)
                    if jt == 3 and h + 1 < NH:
                        R_nxt = emit_R_ic(h + 1, 1, R_state)[0]
                    if last and jt >= 1:
                        emit_M2_mms(own_mps, xtr, es, jt - 1)
                    if last and jt == 7:
                        nc.vector.tensor_add(
                            out=o2acc[:], in0=o2acc[:], in1=xf_sb[:],
                        )

                if not last:
                    mps = pm_psum.tile([CA, SEQ], FP, tag="pm", name=f"mp_{h}")
                    prev = (h, es, xtr, mps)
                R_cur = R_nxt

            # drain the last head's M2 tail (j-tile 7) and final output
            emit_M2_mms(own_mps, xtr, es, 7)
            pm2 = emit_m2_conv(NH - 1, own_mps)
            emit_out2(NH - 1, pm2)

    nc.compile()
    return nc


_CACHE: dict = {}


def _get_nc():
    if "nc" not in _CACHE:
        _CACHE["nc"] = _build()
    return _CACHE["nc"]


def _prep_in_maps(x, W_proj, b_proj, W_out, b_out):
    bf = ml_dtypes.bfloat16
    x = np.ascontiguousarray(np.asarray(x, dtype=np.float32))
    W_proj = np.asarray(W_proj, dtype=np.float32)
    b_proj = np.asarray(b_proj, dtype=np.float32)
    W_out = np.asarray(W_out, dtype=np.float32)
    b_out = np.asarray(b_out, dtype=np.float32)

    x2 = x.reshape(N_CORES, C, SEQ)
    W_QA = SEQ + NH * CA + NH * C

    # augmented per-head projection blocks [65, 640]
    Wa = np.concatenate([W_proj, b_proj[None, :]], axis=0)  # [65, 7680]
    gt = np.empty((CA, NH, CA), dtype=np.float32)
    ffm = np.empty((CA, NH, C), dtype=np.float32)
    for h in range(NH):
        q0 = h * 3 * D
        Wq = Wa[:, q0:q0 + D]            # [65, 640]
        Wk = Wa[:, q0 + D:q0 + 2 * D]
        Wv = Wa[:, q0 + 2 * D:q0 + 3 * D]
        G = Wk @ Wq.T                    # [65, 65]; scoresT = Xa^T G Xa
        gt[:, h, :] = G.T                # lhsT[c', c] = G[c, c']
        ffm[:, h, :] = Wv @ W_out[h * D:(h + 1) * D, :]   # [65, 64]

    # blob 1: xa | gt | ff  on 65 partitions
    qa_all = np.empty((N_CORES, CA, W_QA), dtype=bf)
    qa_all[:, :C, :SEQ] = x2.astype(bf)
    qa_all[:, C, :SEQ] = np.float32(1.0)
    qa_all[:, :, SEQ:SEQ + NH * CA] = gt.reshape(CA, NH * CA).astype(bf)[None]
    qa_all[:, :, SEQ + NH * CA:] = ffm.reshape(CA, NH * C).astype(bf)[None]

    # blob 2: XaT [p, jt*65+c'] = x[c', jt*128+p], ones at c'=64
    xt_all = np.empty((N_CORES, 128, JT, CA), dtype=bf)
    xtt = x2.transpose(0, 2, 1).reshape(N_CORES, JT, 128, C)  # [b, jt, p, c]
    xt_all[:, :, :, :C] = xtt.transpose(0, 2, 1, 3).astype(bf)
    xt_all[:, :, :, C] = np.float32(1.0)
    xt_all = xt_all.reshape(N_CORES, 128, JT * CA)

    # blob 3: xf | b_out in f32
    xr_all = np.empty((N_CORES, C, SEQ + 1), dtype=np.float32)
    xr_all[:, :, :SEQ] = x2
    xr_all[:, :, SEQ] = b_out[None, :]

    return [
        {
            "qa": np.ascontiguousarray(qa_all[i]),
            "xt": np.ascontiguousarray(xt_all[i]),
            "xr": np.ascontiguousarray(xr_all[i]),
        }
        for i in range(N_CORES)
    ]


def run(x, t, W_proj, b_proj, W_out, b_out, trace=False, **trace_kwargs):
    in_maps = _prep_in_maps(x, W_proj, b_proj, W_out, b_out)
    res = run_bass_kernel_spmd(
        _get_nc(), in_maps, core_ids=list(range(N_CORES)),
        trace=trace, **trace_kwargs,
    )
    out = np.stack([res.results[i]["out"] for i in range(N_CORES)])
    return out.reshape(N_CORES, C, 32, 32), res


def kernel(x, t=None, W_proj=None, b_proj=None, W_out=None, b_out=None):
    out, _ = run(x, t, W_proj, b_proj, W_out, b_out, trace=False)
    return out


# revision 27
# speedup vs baseline: 1.0282x; 1.0022x over previous
"""Trainium2 Bass kernel for nn_AttLayer (4-head attention, softmax over queries).

Sharding: data-parallel over batch. 8 batch elements -> 8 NeuronCores, zero
collectives.

Key algebraic restructuring: with C=64 channels the attention is rank-65.
Folding the projections through the score/value contractions (bias rows
appended via the augmented-ones trick):

  R_h       = G_h^T-contracted input               G_h = Wk_aug_h @ Wq_aug_h^T
  scoresT_h = Xa^T R_h                             (= Xa^T G_h Xa, 65 x 65 G)
  es        = exp(SCALE * scoresT)                 row-sum den fused into the
                                                   exp activation (accum_out)
  xtr[j,c]  = XaT[j,c] / den[j]                    reciprocal folded into the
                                                   65-wide transposed input
  M2_h[c,i] = sum_j xtr[j,c] * es[j,i]             (65 x 1024)
  out2     += F_h^T @ M2_h                         F_h = Wv_aug_h @ Wout_h
  out       = out2 + b_out + x

G_h and F_h are computed on the host in f32 (exact). Everything on-chip is
bf16 matmuls with f32 PSUM accumulation; the exp/normalize core is the
critical path (ScalarEngine), so all other work is software-pipelined into
the per-j-tile chain steps of neighboring heads.
"""

import numpy as np
import ml_dtypes

import concourse.tile as tile
from concourse import bacc, mybir
from concourse.bass_utils import run_bass_kernel_spmd

NH = 4          # heads
D = 640         # per-head dim
C = 64          # channels
CA = C + 1      # augmented (ones row)
SEQ = 1024      # 32*32
SCALE = float(D) ** -0.5
N_CORES = 8
FP = mybir.dt.float32
BF = mybir.dt.bfloat16

JT = SEQ // 128     # 8 j-tiles (128 keys each)
IC = SEQ // 512     # 2 i-chunks (512 queries each)

AF = mybir.ActivationFunctionType
ALU = mybir.AluOpType


def _build():
    nc = bacc.Bacc(None, target_bir_lowering=False)
    # packed input blobs: one DMA each (descriptor setup dominates small DMAs)
    W_QA = SEQ + NH * CA + NH * C          # xa | gt | ff   on rows 0..64
    qa = nc.declare_dram_parameter("qa", [CA, W_QA], BF, isOutput=False)
    xt = nc.declare_dram_parameter("xt", [128, JT * CA], BF, isOutput=False)
    xr = nc.declare_dram_parameter("xr", [C, SEQ + 1], FP, isOutput=False)
    out = nc.declare_dram_parameter("out", [C, SEQ], FP, isOutput=True)

    with tile.TileContext(nc) as tc:
        with (
            tc.tile_pool(name="consts", bufs=1) as consts,
            tc.tile_pool(name="hpool", bufs=3) as hpool,
            tc.tile_pool(name="sc", bufs=2, space="PSUM") as sc_psum,
            tc.tile_pool(name="pm", bufs=2, space="PSUM") as pm_psum,
        ):
            qa_sb = consts.tile([CA, W_QA], BF)
            # weights chunk first (gates R), then the two xa halves, each on
            # its own DMA queue
            nc.sync.dma_start(out=qa_sb[:, SEQ:], in_=qa[:, SEQ:])
            for ic in range(IC):
                nc.sync.dma_start(
                    out=qa_sb[:, ic * 512:(ic + 1) * 512],
                    in_=qa[:, ic * 512:(ic + 1) * 512],
                )
            xtb_sb = consts.tile([128, JT * CA], BF)
            xr_sb = consts.tile([C, SEQ + 1], FP)
            xa_sb = qa_sb[:, 0:SEQ]

            def gt_view(h):
                return qa_sb[:, SEQ + h * CA: SEQ + (h + 1) * CA]

            def ff_view(h):
                return qa_sb[:, SEQ + NH * CA + h * C: SEQ + NH * CA + (h + 1) * C]

            def xt_view(jt):
                return xtb_sb[:, jt * CA:(jt + 1) * CA]

            xf_sb = xr_sb[:, 0:SEQ]
            bo_sb = xr_sb[:, SEQ:SEQ + 1]
            out_sb = consts.tile([C, SEQ], FP)
            o2acc = consts.tile([C, SEQ], FP)

            def emit_late_dmas():
                nc.sync.dma_start(out=xtb_sb[:], in_=xt[:, :])
                nc.sync.dma_start(out=xr_sb[:], in_=xr[:, :])

            def emit_R_ic(h, ic, state):
                if ic == 0:
                    state = (
                        hpool.tile([CA, SEQ], BF, tag="R", name=f"R_{h}"),
                        pm_psum.tile([CA, SEQ], FP, tag="pm", name=f"rp_{h}"),
                    )
                R_sb, rps = state
                nc.tensor.matmul(
                    rps[:, ic * 512:(ic + 1) * 512],
                    lhsT=gt_view(h),
                    rhs=xa_sb[:, ic * 512:(ic + 1) * 512],
                    start=True, stop=True,
                )
                nc.vector.tensor_copy(
                    out=R_sb[:, ic * 512:(ic + 1) * 512],
                    in_=rps[:, ic * 512:(ic + 1) * 512],
                )
                return state

            def emit_R(h):
                state = emit_R_ic(h, 0, None)
                state = emit_R_ic(h, 1, state)
                return state[0]

            def emit_M2_mms(mps, xtr, es, jt):
                for ic in range(IC):
                    nc.tensor.matmul(
                        mps[:, ic * 512:(ic + 1) * 512],
                        lhsT=xtr[:, jt, :],
                        rhs=es[:, jt, ic * 512:(ic + 1) * 512],
                        start=(jt == 0), stop=(jt == JT - 1),
                    )

            def emit_m2_conv(ph, pmps):
                pm2 = hpool.tile([CA, SEQ], BF, tag="m2", name=f"m2_{ph}")
                for ic in range(IC):
                    nc.vector.tensor_copy(
                        out=pm2[:, ic * 512:(ic + 1) * 512],
                        in_=pmps[:, ic * 512:(ic + 1) * 512],
                    )
                return pm2

            def emit_out2(h, m2):
                o2p = pm_psum.tile([CA, SEQ], FP, tag="pm", name=f"o2_{h}")
                for ic in range(IC):
                    nc.tensor.matmul(
                        o2p[:C, ic * 512:(ic + 1) * 512],
                        lhsT=ff_view(h),
                        rhs=m2[:, ic * 512:(ic + 1) * 512],
                        start=True, stop=True,
                    )
                if h == 0:
                    nc.vector.tensor_copy(out=o2acc[:], in_=o2p[:C, :])
                elif h < NH - 1:
                    nc.vector.tensor_add(out=o2acc[:], in0=o2acc[:], in1=o2p[:C, :])
                else:
                    # final head: o2acc already holds heads 0-2 plus residual
                    for ic in range(IC):
                        sl = slice(ic * 512, (ic + 1) * 512)
                        nc.scalar.activation(
                            out=out_sb[:, sl],
                            in_=o2p[:C, sl],
                            func=AF.Identity,
                            bias=bo_sb[:],
                            scale=1.0,
                        )
                        nc.vector.tensor_add(
                            out=out_sb[:, sl], in0=out_sb[:, sl], in1=o2acc[:, sl],
                        )
                        for q in range(2):
                            qsl = slice(ic * 512 + q * 256, ic * 512 + (q + 1) * 256)
                            nc.sync.dma_start(out=out[:, qsl], in_=out_sb[:, qsl])

            R_cur = emit_R(0)
            emit_late_dmas()
            R_nxt = None
            prev = None   # (h, es, xtr, mps) of the previous head
            for h in range(NH):
                R_sb = R_cur
                last = h == NH - 1
                es = hpool.tile([128, JT, SEQ], BF, tag="es", name=f"es_{h}")
                xtr = hpool.tile([128, JT, CA], BF, tag="xtr", name=f"xtr_{h}")
                den = hpool.tile([128, JT], FP, tag="den", name=f"den_{h}")
                rec = hpool.tile([128, JT], FP, tag="rec", name=f"rec_{h}")
                own_mps = (
                    pm_psum.tile([CA, SEQ], FP, tag="pm", name="mp_last")
                    if last else None
                )

                for jt in range(JT):
                    pst = sc_psum.tile([128, SEQ], FP, tag="sc", name=f"sc_{h}_{jt}")
                    for ic in range(IC):
                        nc.tensor.matmul(
                            pst[:, ic * 512:(ic + 1) * 512],
                            lhsT=xa_sb[:, jt * 128:(jt + 1) * 128],
                            rhs=R_sb[:, ic * 512:(ic + 1) * 512],
                            start=True, stop=True,
                        )
                    nc.scalar.activation(
                        out=es[:, jt, :],
                        in_=pst[:],
                        func=AF.Exp,
                        scale=SCALE,
                        accum_out=den[:, jt:jt + 1],
                    )
                    nc.vector.reciprocal(out=rec[:, jt:jt + 1], in_=den[:, jt:jt + 1])
                    nc.vector.tensor_scalar_mul(
                        xtr[:, jt, :], xt_view(jt), rec[:, jt:jt + 1],
                    )

                    # ---- pipelined injections (<=2 matmuls per chain step)
                    if prev is not None:
                        ph, pes, pxtr, pmps = prev
                        emit_M2_mms(pmps, pxtr, pes, jt)
                        if jt == JT - 1:
                            pm2 = emit_m2_conv(ph, pmps)
                            emit_out2(ph, pm2)
                            prev = None
                    if jt == 2 and h + 1 < NH:
                        R_state = emit_R_ic(h + 1, 0, None)
                    if jt == 3 and h + 1 < NH:
                        R_nxt = emit_R_ic(h + 1, 1, R_state)[0]
                    if last and jt >= 1:
                        emit_M2_mms(own_mps, xtr, es, jt - 1)
                    if last and jt == 7:
                        nc.vector.tensor_add(
                            out=o2acc[:], in0=o2acc[:], in1=xf_sb[:],
                        )

                if not last:
                    mps = pm_psum.tile([CA, SEQ], FP, tag="pm", name=f"mp_{h}")
                    prev = (h, es, xtr, mps)
                R_cur = R_nxt

            # drain the last head's M2 tail (j-tile 7) and final output
            emit_M2_mms(own_mps, xtr, es, 7)
            pm2 = emit_m2_conv(NH - 1, own_mps)
            emit_out2(NH - 1, pm2)

    nc.compile()
    return nc


_CACHE: dict = {}


def _get_nc():
    if "nc" not in _CACHE:
        _CACHE["nc"] = _build()
    return _CACHE["nc"]


def _prep_in_maps(x, W_proj, b_proj, W_out, b_out):
    bf = ml_dtypes.bfloat16
    x = np.ascontiguousarray(np.asarray(x, dtype=np.float32))
    W_proj = np.asarray(W_proj, dtype=np.float32)
    b_proj = np.asarray(b_proj, dtype=np.float32)
    W_out = np.asarray(W_out, dtype=np.float32)
    b_out = np.asarray(b_out, dtype=np.float32)

    x2 = x.reshape(N_CORES, C, SEQ)
    W_QA = SEQ + NH * CA + NH * C

    # augmented per-head projection blocks [65, 640]
    Wa = np.concatenate([W_proj, b_proj[None, :]], axis=0)  # [65, 7680]
    gt = np.empty((CA, NH, CA), dtype=np.float32)
    ffm = np.empty((CA, NH, C), dtype=np.float32)
    for h in range(NH):
        q0 = h * 3 * D
        Wq = Wa[:, q0:q0 + D]            # [65, 640]
        Wk = Wa[:, q0 + D:q0 + 2 * D]
        Wv = Wa[:, q0 + 2 * D:q0 + 3 * D]
        G = Wk @ Wq.T                    # [65, 65]; scoresT = Xa^T G Xa
        gt[:, h, :] = G.T                # lhsT[c', c] = G[c, c']
        ffm[:, h, :] = Wv @ W_out[h * D:(h + 1) * D, :]   # [65, 64]

    # blob 1: xa | gt | ff  on 65 partitions
    qa_all = np.empty((N_CORES, CA, W_QA), dtype=bf)
    qa_all[:, :C, :SEQ] = x2.astype(bf)
    qa_all[:, C, :SEQ] = np.float32(1.0)
    qa_all[:, :, SEQ:SEQ + NH * CA] = gt.reshape(CA, NH * CA).astype(bf)[None]
    qa_all[:, :, SEQ + NH * CA:] = ffm.reshape(CA, NH * C).astype(bf)[None]

    # blob 2: XaT [p, jt*65+c'] = x[c', jt*128+p], ones at c'=64
    xt_all = np.empty((N_CORES, 128, JT, CA), dtype=bf)
    xtt = x2.transpose(0, 2, 1).reshape(N_CORES, JT, 128, C)  # [b, jt, p, c]
    xt_all[:, :, :, :C] = xtt.transpose(0, 2, 1, 3).astype(bf)
    xt_all[:, :, :, C] = np.float32(1.0)
    xt_all = xt_all.reshape(N_CORES, 128, JT * CA)

    # blob 3: xf | b_out in f32
    xr_all = np.empty((N_CORES, C, SEQ + 1), dtype=np.float32)
    xr_all[:, :, :SEQ] = x2
    xr_all[:, :, SEQ] = b_out[None, :]

    return [
        {
            "qa": np.ascontiguousarray(qa_all[i]),
            "xt": np.ascontiguousarray(xt_all[i]),
            "xr": np.ascontiguousarray(xr_all[i]),
        }
        for i in range(N_CORES)
    ]


def run(x, t, W_proj, b_proj, W_out, b_out, trace=False, **trace_kwargs):
    in_maps = _prep_in_maps(x, W_proj, b_proj, W_out, b_out)
    res = run_bass_kernel_spmd(
        _get_nc(), in_maps, core_ids=list(range(N_CORES)),
        trace=trace, **trace_kwargs,
    )
    out = np.stack([res.results[i]["out"] for i in range(N_CORES)])
    return out.reshape(N_CORES, C, 32, 32), res


def kernel(x, t=None, W_proj=None, b_proj=None, W_out=None, b_out=None):
    out, _ = run(x, t, W_proj, b_proj, W_out, b_out, trace=False)
    return out


# revision 28
# speedup vs baseline: 1.0337x; 1.0053x over previous
"""Trainium2 Bass kernel for nn_AttLayer (4-head attention, softmax over queries).

Sharding: data-parallel over batch. 8 batch elements -> 8 NeuronCores, zero
collectives.

Key algebraic restructuring: with C=64 channels the attention is rank-65.
Folding the projections through the score/value contractions (bias rows
appended via the augmented-ones trick):

  R_h       = G_h^T-contracted input               G_h = Wk_aug_h @ Wq_aug_h^T
  scoresT_h = Xa^T R_h                             (= Xa^T G_h Xa, 65 x 65 G)
  es        = exp(SCALE * scoresT)                 row-sum den fused into the
                                                   exp activation (accum_out)
  xtr[j,c]  = XaT[j,c] / den[j]                    reciprocal folded into the
                                                   65-wide transposed input
  M2_h[c,i] = sum_j xtr[j,c] * es[j,i]             (65 x 1024)
  out2     += F_h^T @ M2_h                         F_h = Wv_aug_h @ Wout_h
  out       = out2 + b_out + x

G_h and F_h are computed on the host in f32 (exact). Everything on-chip is
bf16 matmuls with f32 PSUM accumulation; the exp/normalize core is the
critical path (ScalarEngine), so all other work is software-pipelined into
the per-j-tile chain steps of neighboring heads.
"""

import numpy as np
import ml_dtypes

import concourse.tile as tile
from concourse import bacc, mybir
from concourse.bass_utils import run_bass_kernel_spmd

NH = 4          # heads
D = 640         # per-head dim
C = 64          # channels
CA = C + 1      # augmented (ones row)
SEQ = 1024      # 32*32
SCALE = float(D) ** -0.5
N_CORES = 8
FP = mybir.dt.float32
BF = mybir.dt.bfloat16

JT = SEQ // 128     # 8 j-tiles (128 keys each)
IC = SEQ // 512     # 2 i-chunks (512 queries each)

AF = mybir.ActivationFunctionType
ALU = mybir.AluOpType


def _build():
    nc = bacc.Bacc(None, target_bir_lowering=False)
    # packed input blobs: one DMA each (descriptor setup dominates small DMAs)
    W_QA = SEQ + NH * CA + NH * C          # xa | gt | ff   on rows 0..64
    qa = nc.declare_dram_parameter("qa", [CA, W_QA], BF, isOutput=False)
    xt = nc.declare_dram_parameter("xt", [128, JT * CA], BF, isOutput=False)
    xr = nc.declare_dram_parameter("xr", [C, SEQ + 1], FP, isOutput=False)
    out = nc.declare_dram_parameter("out", [C, SEQ], FP, isOutput=True)

    with tile.TileContext(nc) as tc:
        with (
            tc.tile_pool(name="consts", bufs=1) as consts,
            tc.tile_pool(name="hpool", bufs=3) as hpool,
            tc.tile_pool(name="sc", bufs=2, space="PSUM") as sc_psum,
            tc.tile_pool(name="pm", bufs=2, space="PSUM") as pm_psum,
        ):
            qa_sb = consts.tile([CA, W_QA], BF)
            # weights chunk first (gates R), then the two xa halves, each on
            # its own DMA queue
            nc.sync.dma_start(out=qa_sb[:, SEQ:], in_=qa[:, SEQ:])
            for ic in range(IC):
                nc.sync.dma_start(
                    out=qa_sb[:, ic * 512:(ic + 1) * 512],
                    in_=qa[:, ic * 512:(ic + 1) * 512],
                )
            xtb_sb = consts.tile([128, JT * CA], BF)
            xr_sb = consts.tile([C, SEQ + 1], FP)
            xa_sb = qa_sb[:, 0:SEQ]

            def gt_view(h):
                return qa_sb[:, SEQ + h * CA: SEQ + (h + 1) * CA]

            def ff_view(h):
                return qa_sb[:, SEQ + NH * CA + h * C: SEQ + NH * CA + (h + 1) * C]

            def xt_view(jt):
                return xtb_sb[:, jt * CA:(jt + 1) * CA]

            xf_sb = xr_sb[:, 0:SEQ]
            bo_sb = xr_sb[:, SEQ:SEQ + 1]
            out_sb = consts.tile([C, SEQ], FP)
            o2acc = consts.tile([C, SEQ], FP)

            def emit_late_dmas():
                nc.sync.dma_start(out=xtb_sb[:], in_=xt[:, :])
                nc.sync.dma_start(out=xr_sb[:], in_=xr[:, :])

            def emit_R_ic(h, ic, state):
                if ic == 0:
                    state = (
                        hpool.tile([CA, SEQ], BF, tag="R", name=f"R_{h}"),
                        pm_psum.tile([CA, SEQ], FP, tag="pm", name=f"rp_{h}"),
                    )
                R_sb, rps = state
                nc.tensor.matmul(
                    rps[:, ic * 512:(ic + 1) * 512],
                    lhsT=gt_view(h),
                    rhs=xa_sb[:, ic * 512:(ic + 1) * 512],
                    start=True, stop=True,
                )
                nc.vector.tensor_copy(
                    out=R_sb[:, ic * 512:(ic + 1) * 512],
                    in_=rps[:, ic * 512:(ic + 1) * 512],
                )
                return state

            def emit_R(h):
                state = emit_R_ic(h, 0, None)
                state = emit_R_ic(h, 1, state)
                return state[0]

            def emit_M2_mms(mps, xtr, es, jt):
                for ic in range(IC):
                    nc.tensor.matmul(
                        mps[:, ic * 512:(ic + 1) * 512],
                        lhsT=xtr[:, jt, :],
                        rhs=es[:, jt, ic * 512:(ic + 1) * 512],
                        start=(jt == 0), stop=(jt == JT - 1),
                    )

            def emit_m2_conv(ph, pmps):
                pm2 = hpool.tile([CA, SEQ], BF, tag="m2", name=f"m2_{ph}")
                for ic in range(IC):
                    nc.vector.tensor_copy(
                        out=pm2[:, ic * 512:(ic + 1) * 512],
                        in_=pmps[:, ic * 512:(ic + 1) * 512],
                    )
                return pm2

            def emit_out2(h, m2):
                o2p = pm_psum.tile([CA, SEQ], FP, tag="pm", name=f"o2_{h}")
                for ic in range(IC):
                    nc.tensor.matmul(
                        o2p[:C, ic * 512:(ic + 1) * 512],
                        lhsT=ff_view(h),
                        rhs=m2[:, ic * 512:(ic + 1) * 512],
                        start=True, stop=True,
                    )
                if h == 0:
                    nc.vector.tensor_copy(out=o2acc[:], in_=o2p[:C, :])
                elif h < NH - 1:
                    nc.vector.tensor_add(out=o2acc[:], in0=o2acc[:], in1=o2p[:C, :])
                else:
                    # final head: o2acc already holds heads 0-2 plus residual
                    for ic in range(IC):
                        sl = slice(ic * 512, (ic + 1) * 512)
                        nc.scalar.activation(
                            out=out_sb[:, sl],
                            in_=o2p[:C, sl],
                            func=AF.Identity,
                            bias=bo_sb[:],
                            scale=1.0,
                        )
                        nc.vector.tensor_add(
                            out=out_sb[:, sl], in0=out_sb[:, sl], in1=o2acc[:, sl],
                        )
                        for q in range(2):
                            qsl = slice(ic * 512 + q * 256, ic * 512 + (q + 1) * 256)
                            nc.sync.dma_start(out=out[:, qsl], in_=out_sb[:, qsl])

            R_cur = emit_R(0)
            emit_late_dmas()
            R_nxt = None
            prev = None   # (h, es, xtr, mps) of the previous head
            for h in range(NH):
                R_sb = R_cur
                last = h == NH - 1
                es = hpool.tile([128, JT, SEQ], BF, tag="es", name=f"es_{h}")
                xtr = hpool.tile([128, JT, CA], BF, tag="xtr", name=f"xtr_{h}")
                den = hpool.tile([128, JT], FP, tag="den", name=f"den_{h}")
                rec = hpool.tile([128, JT], FP, tag="rec", name=f"rec_{h}")
                own_mps = (
                    pm_psum.tile([CA, SEQ], FP, tag="pm", name="mp_last")
                    if last else None
                )

                for jt in range(JT):
                    pst = sc_psum.tile([128, SEQ], FP, tag="sc", name=f"sc_{h}_{jt}")
                    for ic in range(IC):
                        nc.tensor.matmul(
                            pst[:, ic * 512:(ic + 1) * 512],
                            lhsT=xa_sb[:, jt * 128:(jt + 1) * 128],
                            rhs=R_sb[:, ic * 512:(ic + 1) * 512],
                            start=True, stop=True,
                        )
                    nc.scalar.activation(
                        out=es[:, jt, :],
                        in_=pst[:],
                        func=AF.Exp,
                        scale=SCALE,
                        accum_out=den[:, jt:jt + 1],
                    )
                    nc.vector.reciprocal(out=rec[:, jt:jt + 1], in_=den[:, jt:jt + 1])
                    nc.vector.tensor_scalar_mul(
                        xtr[:, jt, :], xt_view(jt), rec[:, jt:jt + 1],
                    )

                    # ---- pipelined injections (<=2 matmuls per chain step)
                    if prev is not None:
                        ph, pes, pxtr, pmps = prev
                        emit_M2_mms(pmps, pxtr, pes, jt)
                        if jt == JT - 1:
                            pm2 = emit_m2_conv(ph, pmps)
                            emit_out2(ph, pm2)
                            prev = None
                    if jt == 2 and h + 1 < NH:
                        R_state = emit_R_ic(h + 1, 0, None)
                    if jt == 3 and h + 1 < NH:
                        R_nxt = emit_R_ic(h + 1, 1, R_state)[0]
                    if last and jt >= 1:
                        emit_M2_mms(own_mps, xtr, es, jt - 1)
                    if last and jt == 7:
                        nc.vector.tensor_add(
                            out=o2acc[:], in0=o2acc[:], in1=xf_sb[:],
                        )

                if not last:
                    mps = pm_psum.tile([CA, SEQ], FP, tag="pm", name=f"mp_{h}")
                    prev = (h, es, xtr, mps)
                R_cur = R_nxt

            # drain the last head's M2 tail (j-tile 7) and final output,
            # fully per-i-chunk so DVE/PE/ACT/DMA overlap
            emit_M2_mms(own_mps, xtr, es, 7)
            pm2 = hpool.tile([CA, SEQ], BF, tag="m2", name="m2_last")
            o2p = pm_psum.tile([CA, SEQ], FP, tag="pm", name="o2_last")
            for ic in range(IC):
                sl = slice(ic * 512, (ic + 1) * 512)
                nc.vector.tensor_copy(out=pm2[:, sl], in_=own_mps[:, sl])
                nc.tensor.matmul(
                    o2p[:C, sl],
                    lhsT=ff_view(NH - 1),
                    rhs=pm2[:, sl],
                    start=True, stop=True,
                )
                nc.scalar.activation(
                    out=out_sb[:, sl],
                    in_=o2p[:C, sl],
                    func=AF.Identity,
                    bias=bo_sb[:],
                    scale=1.0,
                )
                nc.vector.tensor_add(
                    out=out_sb[:, sl], in0=out_sb[:, sl], in1=o2acc[:, sl],
                )
                for q in range(2):
                    qsl = slice(ic * 512 + q * 256, ic * 512 + (q + 1) * 256)
                    nc.sync.dma_start(out=out[:, qsl], in_=out_sb[:, qsl])

    nc.compile()
    return nc


_CACHE: dict = {}


def _get_nc():
    if "nc" not in _CACHE:
        _CACHE["nc"] = _build()
    return _CACHE["nc"]


def _prep_in_maps(x, W_proj, b_proj, W_out, b_out):
    bf = ml_dtypes.bfloat16
    x = np.ascontiguousarray(np.asarray(x, dtype=np.float32))
    W_proj = np.asarray(W_proj, dtype=np.float32)
    b_proj = np.asarray(b_proj, dtype=np.float32)
    W_out = np.asarray(W_out, dtype=np.float32)
    b_out = np.asarray(b_out, dtype=np.float32)

    x2 = x.reshape(N_CORES, C, SEQ)
    W_QA = SEQ + NH * CA + NH * C

    # augmented per-head projection blocks [65, 640]
    Wa = np.concatenate([W_proj, b_proj[None, :]], axis=0)  # [65, 7680]
    gt = np.empty((CA, NH, CA), dtype=np.float32)
    ffm = np.empty((CA, NH, C), dtype=np.float32)
    for h in range(NH):
        q0 = h * 3 * D
        Wq = Wa[:, q0:q0 + D]            # [65, 640]
        Wk = Wa[:, q0 + D:q0 + 2 * D]
        Wv = Wa[:, q0 + 2 * D:q0 + 3 * D]
        G = Wk @ Wq.T                    # [65, 65]; scoresT = Xa^T G Xa
        gt[:, h, :] = G.T                # lhsT[c', c] = G[c, c']
        ffm[:, h, :] = Wv @ W_out[h * D:(h + 1) * D, :]   # [65, 64]

    # blob 1: xa | gt | ff  on 65 partitions
    qa_all = np.empty((N_CORES, CA, W_QA), dtype=bf)
    qa_all[:, :C, :SEQ] = x2.astype(bf)
    qa_all[:, C, :SEQ] = np.float32(1.0)
    qa_all[:, :, SEQ:SEQ + NH * CA] = gt.reshape(CA, NH * CA).astype(bf)[None]
    qa_all[:, :, SEQ + NH * CA:] = ffm.reshape(CA, NH * C).astype(bf)[None]

    # blob 2: XaT [p, jt*65+c'] = x[c', jt*128+p], ones at c'=64
    xt_all = np.empty((N_CORES, 128, JT, CA), dtype=bf)
    xtt = x2.transpose(0, 2, 1).reshape(N_CORES, JT, 128, C)  # [b, jt, p, c]
    xt_all[:, :, :, :C] = xtt.transpose(0, 2, 1, 3).astype(bf)
    xt_all[:, :, :, C] = np.float32(1.0)
    xt_all = xt_all.reshape(N_CORES, 128, JT * CA)

    # blob 3: xf | b_out in f32
    xr_all = np.empty((N_CORES, C, SEQ + 1), dtype=np.float32)
    xr_all[:, :, :SEQ] = x2
    xr_all[:, :, SEQ] = b_out[None, :]

    return [
        {
            "qa": np.ascontiguousarray(qa_all[i]),
            "xt": np.ascontiguousarray(xt_all[i]),
            "xr": np.ascontiguousarray(xr_all[i]),
        }
        for i in range(N_CORES)
    ]


def run(x, t, W_proj, b_proj, W_out, b_out, trace=False, **trace_kwargs):
    in_maps = _prep_in_maps(x, W_proj, b_proj, W_out, b_out)
    res = run_bass_kernel_spmd(
        _get_nc(), in_maps, core_ids=list(range(N_CORES)),
        trace=trace, **trace_kwargs,
    )
    out = np.stack([res.results[i]["out"] for i in range(N_CORES)])
    return out.reshape(N_CORES, C, 32, 32), res


def kernel(x, t=None, W_proj=None, b_proj=None, W_out=None, b_out=None):
    out, _ = run(x, t, W_proj, b_proj, W_out, b_out, trace=False)
    return out


# revision 29
# speedup vs baseline: 1.0397x; 1.0058x over previous
"""Trainium2 Bass kernel for nn_AttLayer (4-head attention, softmax over queries).

Sharding: data-parallel over batch. 8 batch elements -> 8 NeuronCores, zero
collectives.

Key algebraic restructuring: with C=64 channels the attention is rank-65.
Folding the projections through the score/value contractions (bias rows
appended via the augmented-ones trick):

  R_h       = G_h^T-contracted input               G_h = Wk_aug_h @ Wq_aug_h^T
  scoresT_h = Xa^T R_h                             (= Xa^T G_h Xa, 65 x 65 G)
  es        = exp(SCALE * scoresT)                 row-sum den fused into the
                                                   exp activation (accum_out)
  xtr[j,c]  = XaT[j,c] / den[j]                    reciprocal folded into the
                                                   65-wide transposed input
  M2_h[c,i] = sum_j xtr[j,c] * es[j,i]             (65 x 1024)
  out2     += F_h^T @ M2_h                         F_h = Wv_aug_h @ Wout_h
  out       = out2 + b_out + x

G_h and F_h are computed on the host in f32 (exact). Everything on-chip is
bf16 matmuls with f32 PSUM accumulation; the exp/normalize core is the
critical path (ScalarEngine), so all other work is software-pipelined into
the per-j-tile chain steps of neighboring heads.
"""

import numpy as np
import ml_dtypes

import concourse.tile as tile
from concourse import bacc, mybir
from concourse.bass_utils import run_bass_kernel_spmd

NH = 4          # heads
D = 640         # per-head dim
C = 64          # channels
CA = C + 1      # augmented (ones row)
SEQ = 1024      # 32*32
SCALE = float(D) ** -0.5
N_CORES = 8
FP = mybir.dt.float32
BF = mybir.dt.bfloat16

JT = SEQ // 128     # 8 j-tiles (128 keys each)
IC = SEQ // 512     # 2 i-chunks (512 queries each)

AF = mybir.ActivationFunctionType
ALU = mybir.AluOpType


def _build():
    nc = bacc.Bacc(None, target_bir_lowering=False)
    # packed input blobs: one DMA each (descriptor setup dominates small DMAs)
    W_QA = SEQ + NH * CA + NH * C          # xa | gt | ff   on rows 0..64
    qa = nc.declare_dram_parameter("qa", [CA, W_QA], BF, isOutput=False)
    xt = nc.declare_dram_parameter("xt", [128, JT * CA], BF, isOutput=False)
    xr = nc.declare_dram_parameter("xr", [C, SEQ + 1], FP, isOutput=False)
    out = nc.declare_dram_parameter("out", [C, SEQ], FP, isOutput=True)

    with tile.TileContext(nc) as tc:
        with (
            tc.tile_pool(name="consts", bufs=1) as consts,
            tc.tile_pool(name="hpool", bufs=4) as hpool,
            tc.tile_pool(name="sc", bufs=2, space="PSUM") as sc_psum,
            tc.tile_pool(name="pm", bufs=2, space="PSUM") as pm_psum,
        ):
            qa_sb = consts.tile([CA, W_QA], BF)
            # weights chunk first (gates R), then the two xa halves, each on
            # its own DMA queue
            nc.sync.dma_start(out=qa_sb[:, SEQ:], in_=qa[:, SEQ:])
            for ic in range(IC):
                nc.sync.dma_start(
                    out=qa_sb[:, ic * 512:(ic + 1) * 512],
                    in_=qa[:, ic * 512:(ic + 1) * 512],
                )
            xtb_sb = consts.tile([128, JT * CA], BF)
            xr_sb = consts.tile([C, SEQ + 1], FP)
            xa_sb = qa_sb[:, 0:SEQ]

            def gt_view(h):
                return qa_sb[:, SEQ + h * CA: SEQ + (h + 1) * CA]

            def ff_view(h):
                return qa_sb[:, SEQ + NH * CA + h * C: SEQ + NH * CA + (h + 1) * C]

            def xt_view(jt):
                return xtb_sb[:, jt * CA:(jt + 1) * CA]

            xf_sb = xr_sb[:, 0:SEQ]
            bo_sb = xr_sb[:, SEQ:SEQ + 1]
            out_sb = consts.tile([C, SEQ], FP)
            o2acc = consts.tile([C, SEQ], FP)

            def emit_late_dmas():
                nc.sync.dma_start(out=xtb_sb[:], in_=xt[:, :])
                nc.sync.dma_start(out=xr_sb[:], in_=xr[:, :])

            def emit_R_ic(h, ic, state):
                if ic == 0:
                    state = (
                        hpool.tile([CA, SEQ], BF, tag="R", name=f"R_{h}"),
                        pm_psum.tile([CA, SEQ], FP, tag="pm", name=f"rp_{h}"),
                    )
                R_sb, rps = state
                nc.tensor.matmul(
                    rps[:, ic * 512:(ic + 1) * 512],
                    lhsT=gt_view(h),
                    rhs=xa_sb[:, ic * 512:(ic + 1) * 512],
                    start=True, stop=True,
                )
                nc.vector.tensor_copy(
                    out=R_sb[:, ic * 512:(ic + 1) * 512],
                    in_=rps[:, ic * 512:(ic + 1) * 512],
                )
                return state

            def emit_R(h):
                state = emit_R_ic(h, 0, None)
                state = emit_R_ic(h, 1, state)
                return state[0]

            def emit_M2_mms(mps, xtr, es, jt):
                for ic in range(IC):
                    nc.tensor.matmul(
                        mps[:, ic * 512:(ic + 1) * 512],
                        lhsT=xtr[:, jt, :],
                        rhs=es[:, jt, ic * 512:(ic + 1) * 512],
                        start=(jt == 0), stop=(jt == JT - 1),
                    )

            def emit_m2_conv(ph, pmps):
                pm2 = hpool.tile([CA, SEQ], BF, tag="m2", name=f"m2_{ph}")
                for ic in range(IC):
                    nc.vector.tensor_copy(
                        out=pm2[:, ic * 512:(ic + 1) * 512],
                        in_=pmps[:, ic * 512:(ic + 1) * 512],
                    )
                return pm2

            def emit_out2(h, m2):
                o2p = pm_psum.tile([CA, SEQ], FP, tag="pm", name=f"o2_{h}")
                for ic in range(IC):
                    nc.tensor.matmul(
                        o2p[:C, ic * 512:(ic + 1) * 512],
                        lhsT=ff_view(h),
                        rhs=m2[:, ic * 512:(ic + 1) * 512],
                        start=True, stop=True,
                    )
                if h == 0:
                    nc.vector.tensor_copy(out=o2acc[:], in_=o2p[:C, :])
                elif h < NH - 1:
                    nc.vector.tensor_add(out=o2acc[:], in0=o2acc[:], in1=o2p[:C, :])
                else:
                    # final head: o2acc already holds heads 0-2 plus residual
                    for ic in range(IC):
                        sl = slice(ic * 512, (ic + 1) * 512)
                        nc.scalar.activation(
                            out=out_sb[:, sl],
                            in_=o2p[:C, sl],
                            func=AF.Identity,
                            bias=bo_sb[:],
                            scale=1.0,
                        )
                        nc.vector.tensor_add(
                            out=out_sb[:, sl], in0=out_sb[:, sl], in1=o2acc[:, sl],
                        )
                        for q in range(2):
                            qsl = slice(ic * 512 + q * 256, ic * 512 + (q + 1) * 256)
                            nc.sync.dma_start(out=out[:, qsl], in_=out_sb[:, qsl])

            R_cur = emit_R(0)
            emit_late_dmas()
            R_nxt = None
            prev = None   # (h, es, xtr, mps) of the previous head
            for h in range(NH):
                R_sb = R_cur
                last = h == NH - 1
                es = hpool.tile([128, JT, SEQ], BF, tag="es", name=f"es_{h}")
                xtr = hpool.tile([128, JT, CA], BF, tag="xtr", name=f"xtr_{h}")
                den = hpool.tile([128, JT], FP, tag="den", name=f"den_{h}")
                rec = hpool.tile([128, JT], FP, tag="rec", name=f"rec_{h}")
                own_mps = (
                    pm_psum.tile([CA, SEQ], FP, tag="pm", name="mp_last")
                    if last else None
                )

                for jt in range(JT):
                    pst = sc_psum.tile([128, SEQ], FP, tag="sc", name=f"sc_{h}_{jt}")
                    for ic in range(IC):
                        nc.tensor.matmul(
                            pst[:, ic * 512:(ic + 1) * 512],
                            lhsT=xa_sb[:, jt * 128:(jt + 1) * 128],
                            rhs=R_sb[:, ic * 512:(ic + 1) * 512],
                            start=True, stop=True,
                        )
                    nc.scalar.activation(
                        out=es[:, jt, :],
                        in_=pst[:],
                        func=AF.Exp,
                        scale=SCALE,
                        accum_out=den[:, jt:jt + 1],
                    )
                    nc.vector.reciprocal(out=rec[:, jt:jt + 1], in_=den[:, jt:jt + 1])
                    nc.vector.tensor_scalar_mul(
                        xtr[:, jt, :], xt_view(jt), rec[:, jt:jt + 1],
                    )

                    # ---- pipelined injections (<=2 matmuls per chain step)
                    if prev is not None:
                        ph, pes, pxtr, pmps = prev
                        emit_M2_mms(pmps, pxtr, pes, jt)
                        if jt == JT - 1:
                            pm2 = emit_m2_conv(ph, pmps)
                            emit_out2(ph, pm2)
                            prev = None
                    if jt == 2 and h + 1 < NH:
                        R_state = emit_R_ic(h + 1, 0, None)
                    if jt == 3 and h + 1 < NH:
                        R_nxt = emit_R_ic(h + 1, 1, R_state)[0]
                    if last and jt >= 1:
                        emit_M2_mms(own_mps, xtr, es, jt - 1)
                    if last and jt == 7:
                        nc.vector.tensor_add(
                            out=o2acc[:], in0=o2acc[:], in1=xf_sb[:],
                        )

                if not last:
                    mps = pm_psum.tile([CA, SEQ], FP, tag="pm", name=f"mp_{h}")
                    prev = (h, es, xtr, mps)
                R_cur = R_nxt

            # drain the last head's M2 tail (j-tile 7) and final output,
            # fully per-i-chunk so DVE/PE/ACT/DMA overlap
            emit_M2_mms(own_mps, xtr, es, 7)
            pm2 = hpool.tile([CA, SEQ], BF, tag="m2", name="m2_last")
            o2p = pm_psum.tile([CA, SEQ], FP, tag="pm", name="o2_last")
            for ic in range(IC):
                sl = slice(ic * 512, (ic + 1) * 512)
                nc.vector.tensor_copy(out=pm2[:, sl], in_=own_mps[:, sl])
                nc.tensor.matmul(
                    o2p[:C, sl],
                    lhsT=ff_view(NH - 1),
                    rhs=pm2[:, sl],
                    start=True, stop=True,
                )
                nc.scalar.activation(
                    out=out_sb[:, sl],
                    in_=o2p[:C, sl],
                    func=AF.Identity,
                    bias=bo_sb[:],
                    scale=1.0,
                )
                nc.vector.tensor_add(
                    out=out_sb[:, sl], in0=out_sb[:, sl], in1=o2acc[:, sl],
                )
                for q in range(2):
                    qsl = slice(ic * 512 + q * 256, ic * 512 + (q + 1) * 256)
                    nc.sync.dma_start(out=out[:, qsl], in_=out_sb[:, qsl])

    nc.compile()
    return nc


_CACHE: dict = {}


def _get_nc():
    if "nc" not in _CACHE:
        _CACHE["nc"] = _build()
    return _CACHE["nc"]


def _prep_in_maps(x, W_proj, b_proj, W_out, b_out):
    bf = ml_dtypes.bfloat16
    x = np.ascontiguousarray(np.asarray(x, dtype=np.float32))
    W_proj = np.asarray(W_proj, dtype=np.float32)
    b_proj = np.asarray(b_proj, dtype=np.float32)
    W_out = np.asarray(W_out, dtype=np.float32)
    b_out = np.asarray(b_out, dtype=np.float32)

    x2 = x.reshape(N_CORES, C, SEQ)
    W_QA = SEQ + NH * CA + NH * C

    # augmented per-head projection blocks [65, 640]
    Wa = np.concatenate([W_proj, b_proj[None, :]], axis=0)  # [65, 7680]
    gt = np.empty((CA, NH, CA), dtype=np.float32)
    ffm = np.empty((CA, NH, C), dtype=np.float32)
    for h in range(NH):
        q0 = h * 3 * D
        Wq = Wa[:, q0:q0 + D]            # [65, 640]
        Wk = Wa[:, q0 + D:q0 + 2 * D]
        Wv = Wa[:, q0 + 2 * D:q0 + 3 * D]
        G = Wk @ Wq.T                    # [65, 65]; scoresT = Xa^T G Xa
        gt[:, h, :] = G.T                # lhsT[c', c] = G[c, c']
        ffm[:, h, :] = Wv @ W_out[h * D:(h + 1) * D, :]   # [65, 64]

    # blob 1: xa | gt | ff  on 65 partitions
    qa_all = np.empty((N_CORES, CA, W_QA), dtype=bf)
    qa_all[:, :C, :SEQ] = x2.astype(bf)
    qa_all[:, C, :SEQ] = np.float32(1.0)
    qa_all[:, :, SEQ:SEQ + NH * CA] = gt.reshape(CA, NH * CA).astype(bf)[None]
    qa_all[:, :, SEQ + NH * CA:] = ffm.reshape(CA, NH * C).astype(bf)[None]

    # blob 2: XaT [p, jt*65+c'] = x[c', jt*128+p], ones at c'=64
    xt_all = np.empty((N_CORES, 128, JT, CA), dtype=bf)
    xtt = x2.transpose(0, 2, 1).reshape(N_CORES, JT, 128, C)  # [b, jt, p, c]
    xt_all[:, :, :, :C] = xtt.transpose(0, 2, 1, 3).astype(bf)
    xt_all[:, :, :, C] = np.float32(1.0)
    xt_all = xt_all.reshape(N_CORES, 128, JT * CA)

    # blob 3: xf | b_out in f32
    xr_all = np.empty((N_CORES, C, SEQ + 1), dtype=np.float32)
    xr_all[:, :, :SEQ] = x2
    xr_all[:, :, SEQ] = b_out[None, :]

    return [
        {
            "qa": np.ascontiguousarray(qa_all[i]),
            "xt": np.ascontiguousarray(xt_all[i]),
            "xr": np.ascontiguousarray(xr_all[i]),
        }
        for i in range(N_CORES)
    ]


def run(x, t, W_proj, b_proj, W_out, b_out, trace=False, **trace_kwargs):
    in_maps = _prep_in_maps(x, W_proj, b_proj, W_out, b_out)
    res = run_bass_kernel_spmd(
        _get_nc(), in_maps, core_ids=list(range(N_CORES)),
        trace=trace, **trace_kwargs,
    )
    out = np.stack([res.results[i]["out"] for i in range(N_CORES)])
    return out.reshape(N_CORES, C, 32, 32), res


def kernel(x, t=None, W_proj=None, b_proj=None, W_out=None, b_out=None):
    out, _ = run(x, t, W_proj, b_proj, W_out, b_out, trace=False)
    return out
